# revision 18
# baseline (speedup 1.0000x reference)
"""MoE block (top-1 routing, shared FFN + per-expert LoRA) on 8 TRN2 NeuronCores.

Strategy: data-parallel over the 8192 tokens (1024 tokens/core), weights
replicated. The reference's dense-then-mask expert loop collapses to:

    logits = x @ gate_W.T + gate_b ; e* = argmax(logits)        (fp32)
    u      = x @ A_cat.T                 [N, 32]                (bf16)
    u_m    = u * onehot-mask(e*)  (zero all but selected expert's 4 lora rows)
    inter  = relu(x @ wi_W.T + u_m @ B_cat + wi_b)              (bf16 matmul)
    out    = inter @ wo_W.T + wo_b                              (bf16 matmul)

Everything runs in transposed (feature-major) layout on chip; the host
pre-transposes the shards/weights and re-transposes the output.
"""

import numpy as np
import ml_dtypes
from contextlib import ExitStack

import concourse.bass as bass
import concourse.tile as tile
from concourse import bacc, mybir
from concourse.bass_utils import run_bass_kernel_spmd
from concourse.masks import make_identity

F32 = mybir.dt.float32
F32R = mybir.dt.float32r
BF16 = mybir.dt.bfloat16
U32 = mybir.dt.uint32
BF = ml_dtypes.bfloat16

B, S, D, F, E, R = 4, 2048, 1024, 4096, 8, 4
NCORES = 8
NT = B * S          # 8192 tokens total
N = NT // NCORES    # 1024 tokens per core
ER = E * R          # 32 lora rows
KD = D // 128       # 8 contraction tiles over D
KF = F // 128       # 32 contraction tiles over F
TT = N // 128       # 8 token tiles (routing)
TH = N // 512       # 2 token halves (matmul moving dim)
P = 128

Relu = mybir.ActivationFunctionType.Relu


def _emit(ctx: ExitStack, tc: tile.TileContext, io: dict):
    nc = tc.nc

    consts = ctx.enter_context(tc.tile_pool(name="consts", bufs=1))
    xpool = ctx.enter_context(tc.tile_pool(name="xpool", bufs=1))
    wipool = ctx.enter_context(tc.tile_pool(name="wipool", bufs=1))
    ipool = ctx.enter_context(tc.tile_pool(name="ipool", bufs=1))
    x32p = ctx.enter_context(tc.tile_pool(name="x32p", bufs=3))
    wop = ctx.enter_context(tc.tile_pool(name="wop", bufs=2))
    rwork = ctx.enter_context(tc.tile_pool(name="rwork", bufs=2))
    outp = ctx.enter_context(tc.tile_pool(name="outp", bufs=3))
    sps = ctx.enter_context(tc.tile_pool(name="sps", bufs=1, space="PSUM"))
    bps = ctx.enter_context(tc.tile_pool(name="bps", bufs=4, space="PSUM"))

    # ---------- constants ----------
    identity = consts.tile([P, P], F32, tag="identity")
    make_identity(nc, identity)
    # econst[p, e*R + r] = e  (expert id per lora row, replicated on free axis)
    # cols 32:128 hold an impossible id so the padded mask transposes to zeros
    econst = consts.tile([P, P], BF16, tag="econst")
    for e in range(E):
        nc.vector.memset(econst[:, e * R:(e + 1) * R], float(e))
    nc.vector.memset(econst[:, ER:], 255.0)
    # biases [128, 48] f32: cols 0:32 wi_b by f-tile, 32:40 wo_b by d-tile,
    # 40:48 gate_b replicated. cg [D, 80] bf16: cols 0:8 g16, 8:40 a16,
    # 40:48 dg16, 48:80 da16 (router+lora stationaries, fp32-split).
    biases_sb = consts.tile([P, 48], F32, tag="biases")
    nc.gpsimd.dma_start(out=biases_sb, in_=io["biases"])
    wib_sb = biases_sb[:, 0:KF]
    wob_sb = biases_sb[:, KF:KF + KD]
    gateb_sb = biases_sb[:, KF + KD:KF + KD + E]
    cg_big = consts.tile([P, KD * 80], BF16, tag="cg")
    nc.gpsimd.dma_start(out=cg_big.rearrange("p (k c) -> p k c", k=KD),
                        in_=io["cgT"].rearrange("(k p) c -> p k c", p=P))
    cg_sb = [cg_big[:, k * 80:(k + 1) * 80] for k in range(KD)]
    bcat_sb = consts.tile([ER, F], BF16, tag="bc")
    nc.gpsimd.dma_start(out=bcat_sb, in_=io["bT"])

    # ---------- PE warm-up: release the HAM clock gate while DMAs land ----------
    warm_src = consts.tile([P, 512], BF16, tag="warm")
    nc.vector.memset(warm_src, 1.0)
    for w in range(30):
        psum_w = bps.tile([P, 512], F32, tag="pbig", name=f"pw{w}")
        nc.tensor.matmul(psum_w, lhsT=warm_src[:, 0:P], rhs=warm_src,
                         start=True, stop=True)

    # ---------- resident activations / weights ----------
    inter_sb = [ipool.tile([P, N], BF16, tag=f"inter{f}", name=f"inter{f}")
                for f in range(KF)]
    msc = [consts.tile([P, P], BF16, tag=f"msc{tt}", name=f"msc{tt}")
           for tt in range(TT)]
    um16 = [consts.tile([ER, 512], BF16, tag=f"um{th}", name=f"um{th}")
            for th in range(TH)]

    # ---------- DMA priority order on the sync queue:
    #   x16 -> dx16 (router-critical) -> wi halves -> wo; consts + outs on
    #   the gpsimd queue. Consolidated 3D-AP DMAs to cut issue serialization.
    QF = F // 4   # 1024 f-columns per wi quarter
    x16_big = xpool.tile([P, KD * N], BF16, tag="x16")
    nc.sync.dma_start(out=x16_big.rearrange("p (k t) -> p k t", k=KD),
                      in_=io["xT16"].rearrange("(k p) t -> p k t", p=P))
    x16 = [x16_big[:, k * N:(k + 1) * N] for k in range(KD)]
    wi_src = io["wiT"].rearrange("(k p) f -> p k f", p=P)
    wi_q = []
    for q in range(4):
        wq = wipool.tile([P, KD * QF], BF16, tag=f"wiq{q}", name=f"wiq{q}")
        nc.sync.dma_start(out=wq.rearrange("p (k f) -> p k f", k=KD),
                          in_=wi_src[:, :, q * QF:(q + 1) * QF])
        wi_q.append(wq)
        if q == 0:
            # dx16 (router correction term) rides between wi quarters
            dx16_big = x32p.tile([P, KD * N], BF16, tag="dx16", bufs=1)
            nc.sync.dma_start(out=dx16_big.rearrange("p (k t) -> p k t", k=KD),
                              in_=io["dxT16"].rearrange("(k p) t -> p k t", p=P))
    dx16 = [dx16_big[:, k * N:(k + 1) * N] for k in range(KD)]

    def wi_lhsT(k, f):
        q, fr = divmod(f, 8)
        return wi_q[q][:, k * QF + fr * P:k * QF + (fr + 1) * P]

    # ---------- router + lora projection, one fused group ----------
    # [logits | u] = x@[g | Acat] via 3 bf16 terms (fp32-accurate):
    #   x16@(g16|a16) + dx16@(g16|a16) + x16@(dg16|da16)
    TS = [slice(th * 512, (th + 1) * 512) for th in range(TH)]
    psum_cu = [sps.tile([E + ER, 512], F32, tag=f"pcu{th}", name=f"pcu{th}")
               for th in range(TH)]
    phases = [(0, x16), (40, x16), (0, dx16)]
    for pi, (col, xs) in enumerate(phases):
        for k in range(KD):
            for th in range(TH):
                nc.tensor.matmul(psum_cu[th], lhsT=cg_sb[k][:, col:col + 40],
                                 rhs=xs[k][TS[th]] if isinstance(xs[k], tuple)
                                 else xs[k][:, TS[th]],
                                 start=(pi == 0 and k == 0),
                                 stop=(pi == 2 and k == KD - 1))
    for th in range(TH):
        logitsT = rwork.tile([E, 512], F32, tag="lgT")
        nc.vector.tensor_copy(logitsT, psum_cu[th][ER:ER + E, :])
        for q in range(4):
            tt = th * 4 + q
            # transpose [8, 128] logit chunk to token-major [128, 8]
            psum_tr = sps.tile([P, E], F32, tag="pmask", name=f"ptr{tt}", bufs=2)
            nc.tensor.matmul(psum_tr, lhsT=logitsT[:, q * P:(q + 1) * P],
                             rhs=identity[0:E, 0:E], is_transpose=True,
                             start=True, stop=True)
            logits = rwork.tile([P, E], F32, tag="lg")
            nc.vector.tensor_add(logits, psum_tr, gateb_sb)
            max8 = rwork.tile([P, E], F32, tag="mx")
            nc.vector.max(out=max8, in_=logits)
            idx8 = rwork.tile([P, E], U32, tag="ix")
            nc.vector.max_index(idx8, max8, logits)
            idxf = rwork.tile([P, 1], F32, tag="if")
            nc.vector.tensor_copy(idxf, idx8[:, 0:1])
            # mask_rep[t, e*R+r] = (argmax == e); bf16 is exact for 0/1
            mask_rep = rwork.tile([P, P], BF16, tag="mr")
            nc.vector.tensor_scalar(mask_rep, econst, idxf, None,
                                    mybir.AluOpType.is_equal)
            # transpose token-major -> lora-row-major via DMA (2-byte dtype)
            nc.scalar.dma_start(out=msc[tt], in_=mask_rep, transpose=True)

    # ---------- masked lora projection from the fused psum ----------
    for th in range(TH):
        for q in range(4):
            nc.vector.tensor_mul(um16[th][:, q * P:(q + 1) * P],
                                 psum_cu[th][0:ER, q * P:(q + 1) * P],
                                 msc[th * 4 + q][0:ER, :])

    # ---------- matmul 1: interT = relu(wi @ x.T + Bcat.T @ u_m + wi_b) ------
    # th0/th1 paired per stationary tile so each weight load feeds 2 matmuls
    for f in range(KF):
        ps = [bps.tile([P, 512], F32, tag="pbig", name=f"p1_{f}_{th}")
              for th in range(TH)]
        for k in range(KD):
            for th in range(TH):
                nc.tensor.matmul(ps[th], lhsT=wi_lhsT(k, f),
                                 rhs=x16[k][:, TS[th]],
                                 start=(k == 0), stop=False)
        for th in range(TH):
            nc.tensor.matmul(ps[th], lhsT=bcat_sb[:, f * P:(f + 1) * P],
                             rhs=um16[th], start=False, stop=True)
        for th in range(TH):
            nc.scalar.activation(inter_sb[f][:, TS[th]], ps[th], Relu,
                                 bias=wib_sb[:, f:f + 1])

    # ---------- matmul 2: outT = wo @ inter + wo_b ----------
    # woT [F, D] column-block d fetched as ONE 3D DMA into [p, (kf j)] layout:
    # wo_big[p, kf*128 + j] = woT[kf*128 + p, d*128 + j]
    wo_src = io["woT"].rearrange("(kf p) d -> p kf d", p=P)
    for d in range(KD):
        wo_big = wop.tile([P, F], BF16, tag="wo", name=f"wo{d}")
        nc.sync.dma_start(out=wo_big.rearrange("p (kf j) -> p kf j", kf=KF),
                          in_=wo_src[:, :, d * P:(d + 1) * P])
        ps = [bps.tile([P, 512], F32, tag="pbig", name=f"p2_{d}_{th}")
              for th in range(TH)]
        for kf in range(KF):
            for th in range(TH):
                nc.tensor.matmul(ps[th], lhsT=wo_big[:, kf * P:(kf + 1) * P],
                                 rhs=inter_sb[kf][:, TS[th]],
                                 start=(kf == 0), stop=(kf == KF - 1))
        for th in range(TH):
            osb = outp.tile([P, 512], F32, tag="osb")
            nc.vector.tensor_scalar(osb, ps[th], wob_sb[:, d:d + 1], None,
                                    mybir.AluOpType.add)
            nc.gpsimd.dma_start(out=io["outT"][d * P:(d + 1) * P, TS[th]], in_=osb)


_CACHED_NC = None


def build_nc():
    global _CACHED_NC
    if _CACHED_NC is not None:
        return _CACHED_NC
    nc = bacc.Bacc("TRN2", target_bir_lowering=False, debug=False,
                   enable_asserts=False, num_devices=NCORES)
    decls = [
        ("xT16", [D, N], BF16, False),
        ("dxT16", [D, N], BF16, False),
        ("cgT", [D, 80], BF16, False),
        ("biases", [P, 48], F32, False),
        ("bT", [ER, F], BF16, False),
        ("wiT", [D, F], BF16, False),
        ("woT", [F, D], BF16, False),
        ("outT", [D, N], F32, True),
    ]
    io = {}
    for name, shape, dt_, is_out in decls:
        io[name] = nc.dram_tensor(
            name, shape, dt_, kind="ExternalOutput" if is_out else "ExternalInput"
        ).ap()
    with tile.TileContext(nc) as tc:
        with ExitStack() as ctx:
            _emit(ctx, tc, io)
    nc.compile()
    _CACHED_NC = nc
    return nc


def make_in_maps(inputs: dict) -> list[dict]:
    f32 = np.float32
    x = np.ascontiguousarray(np.asarray(inputs["hidden_states"], f32).reshape(NT, D))
    gT = np.asarray(inputs["gate_W"], f32).T                                # [D, E]
    aT = np.asarray(inputs["lora_A"], f32).reshape(ER, D).T                 # [D, 32]
    ga = np.concatenate([aT, gT], axis=1)                                   # [D, 40]
    ga16 = ga.astype(BF)
    dga16 = (ga - ga16.astype(f32)).astype(BF)
    cgT = np.ascontiguousarray(np.concatenate([ga16, dga16], axis=1))       # [D, 80]
    biases = np.zeros((P, 48), f32)
    biases[:, 0:KF] = np.asarray(inputs["wi_b"], f32).reshape(KF, P).T
    biases[:, KF:KF + KD] = np.asarray(inputs["wo_b"], f32).reshape(KD, P).T
    biases[:, KF + KD:] = np.asarray(inputs["gate_b"], f32)[None, :]
    bT = np.ascontiguousarray(
        np.asarray(inputs["lora_B"], f32).transpose(0, 2, 1).reshape(ER, F).astype(BF))
    wiT = np.ascontiguousarray(np.asarray(inputs["wi_W"], f32).T.astype(BF))  # [D, F]
    woT = np.ascontiguousarray(np.asarray(inputs["wo_W"], f32).T.astype(BF))  # [F, D]

    in_maps = []
    for c in range(NCORES):
        xT32 = np.ascontiguousarray(x[c * N:(c + 1) * N].T)                 # [D, N]
        xT16 = xT32.astype(BF)
        dxT16 = (xT32 - xT16.astype(f32)).astype(BF)
        in_maps.append({
            "xT16": np.ascontiguousarray(xT16),
            "dxT16": np.ascontiguousarray(dxT16),
            "cgT": cgT, "biases": biases, "bT": bT,
            "wiT": wiT, "woT": woT,
        })
    return in_maps


def kernel(**inputs) -> np.ndarray:
    nc = build_nc()
    in_maps = make_in_maps(inputs)
    res = run_bass_kernel_spmd(nc, in_maps, core_ids=list(range(NCORES)))
    out = np.empty((NT, D), np.float32)
    for c in range(NCORES):
        out[c * N:(c + 1) * N] = res.results[c]["outT"].T
    return out.reshape(B, S, D)


# revision 19
# speedup vs baseline: 1.0461x; 1.0461x over previous
"""MoE block (top-1 routing, shared FFN + per-expert LoRA) on 8 TRN2 NeuronCores.

Strategy: data-parallel over the 8192 tokens (1024 tokens/core), weights
replicated. The reference's dense-then-mask expert loop collapses to:

    logits = x @ gate_W.T + gate_b ; e* = argmax(logits)        (fp32)
    u      = x @ A_cat.T                 [N, 32]                (bf16)
    u_m    = u * onehot-mask(e*)  (zero all but selected expert's 4 lora rows)
    inter  = relu(x @ wi_W.T + u_m @ B_cat + wi_b)              (bf16 matmul)
    out    = inter @ wo_W.T + wo_b                              (bf16 matmul)

Everything runs in transposed (feature-major) layout on chip; the host
pre-transposes the shards/weights and re-transposes the output.
"""

import numpy as np
import ml_dtypes
from contextlib import ExitStack

import concourse.bass as bass
import concourse.tile as tile
from concourse import bacc, mybir
from concourse.bass_utils import run_bass_kernel_spmd
from concourse.masks import make_identity

F32 = mybir.dt.float32
F32R = mybir.dt.float32r
BF16 = mybir.dt.bfloat16
U32 = mybir.dt.uint32
BF = ml_dtypes.bfloat16

B, S, D, F, E, R = 4, 2048, 1024, 4096, 8, 4
NCORES = 8
NT = B * S          # 8192 tokens total
N = NT // NCORES    # 1024 tokens per core
ER = E * R          # 32 lora rows
KD = D // 128       # 8 contraction tiles over D
KF = F // 128       # 32 contraction tiles over F
TT = N // 128       # 8 token tiles (routing)
TH = N // 512       # 2 token halves (matmul moving dim)
P = 128

Relu = mybir.ActivationFunctionType.Relu


def _emit(ctx: ExitStack, tc: tile.TileContext, io: dict):
    nc = tc.nc

    consts = ctx.enter_context(tc.tile_pool(name="consts", bufs=1))
    xpool = ctx.enter_context(tc.tile_pool(name="xpool", bufs=1))
    wipool = ctx.enter_context(tc.tile_pool(name="wipool", bufs=1))
    ipool = ctx.enter_context(tc.tile_pool(name="ipool", bufs=1))
    x32p = ctx.enter_context(tc.tile_pool(name="x32p", bufs=3))
    wop = ctx.enter_context(tc.tile_pool(name="wop", bufs=2))
    rwork = ctx.enter_context(tc.tile_pool(name="rwork", bufs=2))
    outp = ctx.enter_context(tc.tile_pool(name="outp", bufs=3))
    sps = ctx.enter_context(tc.tile_pool(name="sps", bufs=1, space="PSUM"))
    bps = ctx.enter_context(tc.tile_pool(name="bps", bufs=4, space="PSUM"))

    # ---------- constants ----------
    identity = consts.tile([P, P], F32, tag="identity")
    make_identity(nc, identity)
    identity16 = consts.tile([P, P], BF16, tag="identity16")
    nc.vector.tensor_copy(identity16, identity)
    # econst[p, e*R + r] = e  (expert id per lora row, replicated on free axis)
    # cols 32:128 hold an impossible id so the padded mask transposes to zeros
    econst = consts.tile([P, P], BF16, tag="econst")
    for e in range(E):
        nc.vector.memset(econst[:, e * R:(e + 1) * R], float(e))
    nc.vector.memset(econst[:, ER:], 255.0)
    # biases [128, 48] f32: cols 0:32 wi_b by f-tile, 32:40 wo_b by d-tile,
    # 40:48 gate_b replicated. cg [D, 80] bf16: cols 0:8 g16, 8:40 a16,
    # 40:48 dg16, 48:80 da16 (router+lora stationaries, fp32-split).
    biases_sb = consts.tile([P, 48], F32, tag="biases")
    nc.gpsimd.dma_start(out=biases_sb, in_=io["biases"])
    wib_sb = biases_sb[:, 0:KF]
    wob_sb = biases_sb[:, KF:KF + KD]
    gateb_sb = biases_sb[:, KF + KD:KF + KD + E]
    cg_big = consts.tile([P, KD * 80], BF16, tag="cg")
    nc.gpsimd.dma_start(out=cg_big.rearrange("p (k c) -> p k c", k=KD),
                        in_=io["cgT"].rearrange("(k p) c -> p k c", p=P))
    cg_sb = [cg_big[:, k * 80:(k + 1) * 80] for k in range(KD)]
    bcat_sb = consts.tile([ER, F], BF16, tag="bc")
    nc.gpsimd.dma_start(out=bcat_sb, in_=io["bT"])

    # ---------- PE warm-up: release the HAM clock gate while DMAs land ----------
    warm_src = consts.tile([P, 512], BF16, tag="warm")
    nc.vector.memset(warm_src, 1.0)
    for w in range(30):
        psum_w = bps.tile([P, 512], F32, tag="pbig", name=f"pw{w}")
        nc.tensor.matmul(psum_w, lhsT=warm_src[:, 0:P], rhs=warm_src,
                         start=True, stop=True)

    # ---------- resident activations / weights ----------
    inter_sb = [ipool.tile([P, N], BF16, tag=f"inter{f}", name=f"inter{f}")
                for f in range(KF)]
    maskT4 = consts.tile([ER, N], BF16, tag="maskT4")
    um16 = [consts.tile([ER, 512], BF16, tag=f"um{th}", name=f"um{th}")
            for th in range(TH)]

    # ---------- DMA priority order on the sync queue:
    #   x16 -> dx16 (router-critical) -> wi halves -> wo; consts + outs on
    #   the gpsimd queue. Consolidated 3D-AP DMAs to cut issue serialization.
    QF = F // 4   # 1024 f-columns per wi quarter
    x16_big = xpool.tile([P, KD * N], BF16, tag="x16")
    nc.sync.dma_start(out=x16_big.rearrange("p (k t) -> p k t", k=KD),
                      in_=io["xT16"].rearrange("(k p) t -> p k t", p=P))
    x16 = [x16_big[:, k * N:(k + 1) * N] for k in range(KD)]
    wi_src = io["wiT"].rearrange("(k p) f -> p k f", p=P)
    wi_q = []
    for q in range(4):
        wq = wipool.tile([P, KD * QF], BF16, tag=f"wiq{q}", name=f"wiq{q}")
        nc.sync.dma_start(out=wq.rearrange("p (k f) -> p k f", k=KD),
                          in_=wi_src[:, :, q * QF:(q + 1) * QF])
        wi_q.append(wq)
        if q == 0:
            # dx16 (router correction term) rides between wi quarters
            dx16_big = x32p.tile([P, KD * N], BF16, tag="dx16", bufs=1)
            nc.sync.dma_start(out=dx16_big.rearrange("p (k t) -> p k t", k=KD),
                              in_=io["dxT16"].rearrange("(k p) t -> p k t", p=P))
    dx16 = [dx16_big[:, k * N:(k + 1) * N] for k in range(KD)]

    def wi_lhsT(k, f):
        q, fr = divmod(f, 8)
        return wi_q[q][:, k * QF + fr * P:k * QF + (fr + 1) * P]

    # ---------- router + lora projection, one fused group ----------
    # [logits | u] = x@[g | Acat] via 3 bf16 terms (fp32-accurate):
    #   x16@(g16|a16) + dx16@(g16|a16) + x16@(dg16|da16)
    TS = [slice(th * 512, (th + 1) * 512) for th in range(TH)]
    psum_cu = [sps.tile([E + ER, 512], F32, tag=f"pcu{th}", name=f"pcu{th}")
               for th in range(TH)]
    phases = [(0, x16), (40, x16), (0, dx16)]
    for pi, (col, xs) in enumerate(phases):
        for k in range(KD):
            for th in range(TH):
                nc.tensor.matmul(psum_cu[th], lhsT=cg_sb[k][:, col:col + 40],
                                 rhs=xs[k][TS[th]] if isinstance(xs[k], tuple)
                                 else xs[k][:, TS[th]],
                                 start=(pi == 0 and k == 0),
                                 stop=(pi == 2 and k == KD - 1))
    for th in range(TH):
        logitsT = rwork.tile([E, 512], F32, tag="lgT")
        nc.vector.tensor_copy(logitsT, psum_cu[th][ER:ER + E, :])
        for q in range(4):
            tt = th * 4 + q
            # transpose [8, 128] logit chunk to token-major [128, 8]
            psum_tr = sps.tile([P, E], F32, tag="pmask", name=f"ptr{tt}", bufs=2)
            nc.tensor.matmul(psum_tr, lhsT=logitsT[:, q * P:(q + 1) * P],
                             rhs=identity[0:E, 0:E], is_transpose=True,
                             start=True, stop=True)
            logits = rwork.tile([P, E], F32, tag="lg")
            nc.vector.tensor_add(logits, psum_tr, gateb_sb)
            max8 = rwork.tile([P, E], F32, tag="mx")
            nc.vector.max(out=max8, in_=logits)
            idx8 = rwork.tile([P, E], U32, tag="ix")
            nc.vector.max_index(idx8, max8, logits)
            idxf = rwork.tile([P, 1], F32, tag="if")
            nc.vector.tensor_copy(idxf, idx8[:, 0:1])
            # mask_rep[t, e*R+r] = (argmax == e); bf16 is exact for 0/1
            mask_rep = rwork.tile([P, ER], BF16, tag="mr")
            nc.vector.tensor_scalar(mask_rep, econst[:, 0:ER], idxf, None,
                                    mybir.AluOpType.is_equal)
            psum_m = sps.tile([ER, P], BF16, tag="pmask", name=f"pm{tt}",
                              bufs=2)
            nc.tensor.matmul(psum_m, lhsT=mask_rep, rhs=identity16,
                             is_transpose=True, start=True, stop=True)
            nc.vector.tensor_copy(maskT4[:, tt * P:(tt + 1) * P], psum_m)

    # ---------- masked lora projection from the fused psum ----------
    for th in range(TH):
        nc.vector.tensor_mul(um16[th], psum_cu[th][0:ER, :],
                             maskT4[:, TS[th]])

    # ---------- matmul 1: interT = relu(wi @ x.T + Bcat.T @ u_m + wi_b) ------
    # th0/th1 paired per stationary tile so each weight load feeds 2 matmuls
    for f in range(KF):
        ps = [bps.tile([P, 512], F32, tag="pbig", name=f"p1_{f}_{th}")
              for th in range(TH)]
        for k in range(KD):
            for th in range(TH):
                nc.tensor.matmul(ps[th], lhsT=wi_lhsT(k, f),
                                 rhs=x16[k][:, TS[th]],
                                 start=(k == 0), stop=False)
        for th in range(TH):
            nc.tensor.matmul(ps[th], lhsT=bcat_sb[:, f * P:(f + 1) * P],
                             rhs=um16[th], start=False, stop=True)
        for th in range(TH):
            nc.scalar.activation(inter_sb[f][:, TS[th]], ps[th], Relu,
                                 bias=wib_sb[:, f:f + 1])

    # ---------- matmul 2: outT = wo @ inter + wo_b ----------
    # woT [F, D] column-block d fetched as ONE 3D DMA into [p, (kf j)] layout:
    # wo_big[p, kf*128 + j] = woT[kf*128 + p, d*128 + j]
    wo_src = io["woT"].rearrange("(kf p) d -> p kf d", p=P)
    for d in range(KD):
        wo_big = wop.tile([P, F], BF16, tag="wo", name=f"wo{d}")
        nc.sync.dma_start(out=wo_big.rearrange("p (kf j) -> p kf j", kf=KF),
                          in_=wo_src[:, :, d * P:(d + 1) * P])
        ps = [bps.tile([P, 512], F32, tag="pbig", name=f"p2_{d}_{th}")
              for th in range(TH)]
        for kf in range(KF):
            for th in range(TH):
                nc.tensor.matmul(ps[th], lhsT=wo_big[:, kf * P:(kf + 1) * P],
                                 rhs=inter_sb[kf][:, TS[th]],
                                 start=(kf == 0), stop=(kf == KF - 1))
        for th in range(TH):
            osb = outp.tile([P, 512], F32, tag="osb")
            nc.vector.tensor_scalar(osb, ps[th], wob_sb[:, d:d + 1], None,
                                    mybir.AluOpType.add)
            nc.gpsimd.dma_start(out=io["outT"][d * P:(d + 1) * P, TS[th]], in_=osb)


_CACHED_NC = None


def build_nc():
    global _CACHED_NC
    if _CACHED_NC is not None:
        return _CACHED_NC
    nc = bacc.Bacc("TRN2", target_bir_lowering=False, debug=False,
                   enable_asserts=False, num_devices=NCORES)
    decls = [
        ("xT16", [D, N], BF16, False),
        ("dxT16", [D, N], BF16, False),
        ("cgT", [D, 80], BF16, False),
        ("biases", [P, 48], F32, False),
        ("bT", [ER, F], BF16, False),
        ("wiT", [D, F], BF16, False),
        ("woT", [F, D], BF16, False),
        ("outT", [D, N], F32, True),
    ]
    io = {}
    for name, shape, dt_, is_out in decls:
        io[name] = nc.dram_tensor(
            name, shape, dt_, kind="ExternalOutput" if is_out else "ExternalInput"
        ).ap()
    with tile.TileContext(nc) as tc:
        with ExitStack() as ctx:
            _emit(ctx, tc, io)
    nc.compile()
    _CACHED_NC = nc
    return nc


def make_in_maps(inputs: dict) -> list[dict]:
    f32 = np.float32
    x = np.ascontiguousarray(np.asarray(inputs["hidden_states"], f32).reshape(NT, D))
    gT = np.asarray(inputs["gate_W"], f32).T                                # [D, E]
    aT = np.asarray(inputs["lora_A"], f32).reshape(ER, D).T                 # [D, 32]
    ga = np.concatenate([aT, gT], axis=1)                                   # [D, 40]
    ga16 = ga.astype(BF)
    dga16 = (ga - ga16.astype(f32)).astype(BF)
    cgT = np.ascontiguousarray(np.concatenate([ga16, dga16], axis=1))       # [D, 80]
    biases = np.zeros((P, 48), f32)
    biases[:, 0:KF] = np.asarray(inputs["wi_b"], f32).reshape(KF, P).T
    biases[:, KF:KF + KD] = np.asarray(inputs["wo_b"], f32).reshape(KD, P).T
    biases[:, KF + KD:] = np.asarray(inputs["gate_b"], f32)[None, :]
    bT = np.ascontiguousarray(
        np.asarray(inputs["lora_B"], f32).transpose(0, 2, 1).reshape(ER, F).astype(BF))
    wiT = np.ascontiguousarray(np.asarray(inputs["wi_W"], f32).T.astype(BF))  # [D, F]
    woT = np.ascontiguousarray(np.asarray(inputs["wo_W"], f32).T.astype(BF))  # [F, D]

    in_maps = []
    for c in range(NCORES):
        xT32 = np.ascontiguousarray(x[c * N:(c + 1) * N].T)                 # [D, N]
        xT16 = xT32.astype(BF)
        dxT16 = (xT32 - xT16.astype(f32)).astype(BF)
        in_maps.append({
            "xT16": np.ascontiguousarray(xT16),
            "dxT16": np.ascontiguousarray(dxT16),
            "cgT": cgT, "biases": biases, "bT": bT,
            "wiT": wiT, "woT": woT,
        })
    return in_maps


def kernel(**inputs) -> np.ndarray:
    nc = build_nc()
    in_maps = make_in_maps(inputs)
    res = run_bass_kernel_spmd(nc, in_maps, core_ids=list(range(NCORES)))
    out = np.empty((NT, D), np.float32)
    for c in range(NCORES):
        out[c * N:(c + 1) * N] = res.results[c]["outT"].T
    return out.reshape(B, S, D)


# revision 20
# speedup vs baseline: 1.0702x; 1.0230x over previous
"""MoE block (top-1 routing, shared FFN + per-expert LoRA) on 8 TRN2 NeuronCores.

Strategy: data-parallel over the 8192 tokens (1024 tokens/core), weights
replicated. The reference's dense-then-mask expert loop collapses to:

    logits = x @ gate_W.T + gate_b ; e* = argmax(logits)        (fp32)
    u      = x @ A_cat.T                 [N, 32]                (bf16)
    u_m    = u * onehot-mask(e*)  (zero all but selected expert's 4 lora rows)
    inter  = relu(x @ wi_W.T + u_m @ B_cat + wi_b)              (bf16 matmul)
    out    = inter @ wo_W.T + wo_b                              (bf16 matmul)

Everything runs in transposed (feature-major) layout on chip; the host
pre-transposes the shards/weights and re-transposes the output.
"""

import numpy as np
import ml_dtypes
from contextlib import ExitStack

import concourse.bass as bass
import concourse.tile as tile
from concourse import bacc, mybir
from concourse.bass_utils import run_bass_kernel_spmd
from concourse.masks import make_identity

F32 = mybir.dt.float32
F32R = mybir.dt.float32r
BF16 = mybir.dt.bfloat16
U32 = mybir.dt.uint32
BF = ml_dtypes.bfloat16

B, S, D, F, E, R = 4, 2048, 1024, 4096, 8, 4
NCORES = 8
NT = B * S          # 8192 tokens total
N = NT // NCORES    # 1024 tokens per core
ER = E * R          # 32 lora rows
KD = D // 128       # 8 contraction tiles over D
KF = F // 128       # 32 contraction tiles over F
TT = N // 128       # 8 token tiles (routing)
TH = N // 512       # 2 token halves (matmul moving dim)
P = 128

Relu = mybir.ActivationFunctionType.Relu


def _emit(ctx: ExitStack, tc: tile.TileContext, io: dict):
    nc = tc.nc

    consts = ctx.enter_context(tc.tile_pool(name="consts", bufs=1))
    xpool = ctx.enter_context(tc.tile_pool(name="xpool", bufs=1))
    wipool = ctx.enter_context(tc.tile_pool(name="wipool", bufs=1))
    ipool = ctx.enter_context(tc.tile_pool(name="ipool", bufs=1))
    x32p = ctx.enter_context(tc.tile_pool(name="x32p", bufs=3))
    wop = ctx.enter_context(tc.tile_pool(name="wop", bufs=2))
    rwork = ctx.enter_context(tc.tile_pool(name="rwork", bufs=2))
    outp = ctx.enter_context(tc.tile_pool(name="outp", bufs=3))
    sps = ctx.enter_context(tc.tile_pool(name="sps", bufs=1, space="PSUM"))
    bps = ctx.enter_context(tc.tile_pool(name="bps", bufs=4, space="PSUM"))

    # ---------- constants ----------
    identity = consts.tile([P, P], F32, tag="identity")
    make_identity(nc, identity)
    identity16 = consts.tile([P, P], BF16, tag="identity16")
    nc.vector.tensor_copy(identity16, identity)
    # econst[p, e*R + r] = e  (expert id per lora row, replicated on free axis)
    # cols 32:128 hold an impossible id so the padded mask transposes to zeros
    econst = consts.tile([P, P], BF16, tag="econst")
    for e in range(E):
        nc.vector.memset(econst[:, e * R:(e + 1) * R], float(e))
    nc.vector.memset(econst[:, ER:], 255.0)
    # biases [128, 48] f32: cols 0:32 wi_b by f-tile, 32:40 wo_b by d-tile,
    # 40:48 gate_b replicated. cg [D, 80] bf16: cols 0:8 g16, 8:40 a16,
    # 40:48 dg16, 48:80 da16 (router+lora stationaries, fp32-split).
    biases_sb = consts.tile([P, 48], F32, tag="biases")
    nc.gpsimd.dma_start(out=biases_sb, in_=io["biases"])
    wib_sb = biases_sb[:, 0:KF]
    wob_sb = biases_sb[:, KF:KF + KD]
    gateb_sb = biases_sb[:, KF + KD:KF + KD + E]
    cg_big = consts.tile([P, KD * 80], BF16, tag="cg")
    nc.gpsimd.dma_start(out=cg_big.rearrange("p (k c) -> p k c", k=KD),
                        in_=io["cgT"].rearrange("(k p) c -> p k c", p=P))
    cg_sb = [cg_big[:, k * 80:(k + 1) * 80] for k in range(KD)]
    bcat_sb = consts.tile([ER, F], BF16, tag="bc")
    nc.gpsimd.dma_start(out=bcat_sb, in_=io["bT"])

    # ---------- PE warm-up: release the HAM clock gate while DMAs land ----------
    warm_src = consts.tile([P, 512], BF16, tag="warm")
    nc.vector.memset(warm_src, 1.0)
    for w in range(30):
        psum_w = bps.tile([P, 512], F32, tag="pbig", name=f"pw{w}")
        nc.tensor.matmul(psum_w, lhsT=warm_src[:, 0:P], rhs=warm_src,
                         start=True, stop=True)

    # ---------- resident activations / weights ----------
    inter_sb = [ipool.tile([P, N], BF16, tag=f"inter{f}", name=f"inter{f}")
                for f in range(KF)]
    maskT4 = consts.tile([ER, N], BF16, tag="maskT4")
    um16 = [consts.tile([ER, 512], BF16, tag=f"um{th}", name=f"um{th}")
            for th in range(TH)]

    # ---------- DMA priority order on the sync queue:
    #   x16 -> dx16 (router-critical) -> wi halves -> wo; consts + outs on
    #   the gpsimd queue. Consolidated 3D-AP DMAs to cut issue serialization.
    QF = F // 4   # 1024 f-columns per wi quarter
    x16_big = xpool.tile([P, KD * N], BF16, tag="x16")
    nc.sync.dma_start(out=x16_big.rearrange("p (k t) -> p k t", k=KD),
                      in_=io["xT16"].rearrange("(k p) t -> p k t", p=P))
    x16 = [x16_big[:, k * N:(k + 1) * N] for k in range(KD)]
    wi_src = io["wiT"].rearrange("(k p) f -> p k f", p=P)
    dx16_big = x32p.tile([P, KD * N], BF16, tag="dx16", bufs=1)
    nc.sync.dma_start(out=dx16_big.rearrange("p (k t) -> p k t", k=KD),
                      in_=io["dxT16"].rearrange("(k p) t -> p k t", p=P))
    dx16 = [dx16_big[:, k * N:(k + 1) * N] for k in range(KD)]
    wi_q = []
    for q in range(4):
        wq = wipool.tile([P, KD * QF], BF16, tag=f"wiq{q}", name=f"wiq{q}")
        nc.sync.dma_start(out=wq.rearrange("p (k f) -> p k f", k=KD),
                          in_=wi_src[:, :, q * QF:(q + 1) * QF])
        wi_q.append(wq)

    def wi_lhsT(k, f):
        q, fr = divmod(f, 8)
        return wi_q[q][:, k * QF + fr * P:k * QF + (fr + 1) * P]

    # ---------- router + lora projection, one fused group ----------
    # [logits | u] = x@[g | Acat] via 3 bf16 terms (fp32-accurate):
    #   x16@(g16|a16) + dx16@(g16|a16) + x16@(dg16|da16)
    TS = [slice(th * 512, (th + 1) * 512) for th in range(TH)]
    psum_cu = [sps.tile([E + ER, 512], F32, tag=f"pcu{th}", name=f"pcu{th}")
               for th in range(TH)]
    gbrow = consts.tile([1, 40], BF16, tag="gbrow")
    nc.vector.memset(gbrow[:, 0:ER], 0.0)
    nc.vector.tensor_copy(gbrow[:, ER:40], gateb_sb[0:1, :])
    ones_row = consts.tile([1, 512], BF16, tag="ones_row")
    nc.vector.memset(ones_row, 1.0)
    phases = [(0, x16), (40, x16), (0, dx16)]
    for pi, (col, xs) in enumerate(phases):
        for k in range(KD):
            for th in range(TH):
                nc.tensor.matmul(psum_cu[th], lhsT=cg_sb[k][:, col:col + 40],
                                 rhs=xs[k][:, TS[th]],
                                 start=(pi == 0 and k == 0), stop=False)
    for th in range(TH):
        nc.tensor.matmul(psum_cu[th], lhsT=gbrow, rhs=ones_row,
                         start=False, stop=True)
    for th in range(TH):
        logitsT = rwork.tile([E, 512], F32, tag="lgT")
        nc.vector.tensor_copy(logitsT, psum_cu[th][ER:ER + E, :])
        for q in range(4):
            tt = th * 4 + q
            # transpose [8, 128] logit chunk to token-major [128, 8]
            psum_tr = sps.tile([P, E], F32, tag="pmask", name=f"ptr{tt}", bufs=2)
            nc.tensor.matmul(psum_tr, lhsT=logitsT[:, q * P:(q + 1) * P],
                             rhs=identity[0:E, 0:E], is_transpose=True,
                             start=True, stop=True)
            logits = rwork.tile([P, E], F32, tag="lg")
            nc.vector.tensor_copy(logits, psum_tr)
            max8 = rwork.tile([P, E], F32, tag="mx")
            nc.vector.max(out=max8, in_=logits)
            idx8 = rwork.tile([P, E], U32, tag="ix")
            nc.vector.max_index(idx8, max8, logits)
            idxf = rwork.tile([P, 1], F32, tag="if")
            nc.vector.tensor_copy(idxf, idx8[:, 0:1])
            # mask_rep[t, e*R+r] = (argmax == e); bf16 is exact for 0/1
            mask_rep = rwork.tile([P, ER], BF16, tag="mr")
            nc.vector.tensor_scalar(mask_rep, econst[:, 0:ER], idxf, None,
                                    mybir.AluOpType.is_equal)
            psum_m = sps.tile([ER, P], BF16, tag="pmask", name=f"pm{tt}",
                              bufs=2)
            nc.tensor.matmul(psum_m, lhsT=mask_rep, rhs=identity16,
                             is_transpose=True, start=True, stop=True)
            nc.vector.tensor_copy(maskT4[:, tt * P:(tt + 1) * P], psum_m)

    # ---------- masked lora projection from the fused psum ----------
    for th in range(TH):
        nc.vector.tensor_mul(um16[th], psum_cu[th][0:ER, :],
                             maskT4[:, TS[th]])

    # ---------- matmul 1: interT = relu(wi @ x.T + Bcat.T @ u_m + wi_b) ------
    # th0/th1 paired per stationary tile so each weight load feeds 2 matmuls
    for f in range(KF):
        ps = [bps.tile([P, 512], F32, tag="pbig", name=f"p1_{f}_{th}")
              for th in range(TH)]
        for k in range(KD):
            for th in range(TH):
                nc.tensor.matmul(ps[th], lhsT=wi_lhsT(k, f),
                                 rhs=x16[k][:, TS[th]],
                                 start=(k == 0), stop=False)
        for th in range(TH):
            nc.tensor.matmul(ps[th], lhsT=bcat_sb[:, f * P:(f + 1) * P],
                             rhs=um16[th], start=False, stop=True)
        for th in range(TH):
            nc.scalar.activation(inter_sb[f][:, TS[th]], ps[th], Relu,
                                 bias=wib_sb[:, f:f + 1])

    # ---------- matmul 2: outT = wo @ inter + wo_b ----------
    # woT [F, D] column-block d fetched as ONE 3D DMA into [p, (kf j)] layout:
    # wo_big[p, kf*128 + j] = woT[kf*128 + p, d*128 + j]
    wo_src = io["woT"].rearrange("(kf p) d -> p kf d", p=P)
    for d in range(KD):
        wo_big = wop.tile([P, F], BF16, tag="wo", name=f"wo{d}")
        nc.sync.dma_start(out=wo_big.rearrange("p (kf j) -> p kf j", kf=KF),
                          in_=wo_src[:, :, d * P:(d + 1) * P])
        ps = [bps.tile([P, 512], F32, tag="pbig", name=f"p2_{d}_{th}")
              for th in range(TH)]
        for kf in range(KF):
            for th in range(TH):
                nc.tensor.matmul(ps[th], lhsT=wo_big[:, kf * P:(kf + 1) * P],
                                 rhs=inter_sb[kf][:, TS[th]],
                                 start=(kf == 0), stop=(kf == KF - 1))
        for th in range(TH):
            osb = outp.tile([P, 512], F32, tag="osb")
            nc.vector.tensor_scalar(osb, ps[th], wob_sb[:, d:d + 1], None,
                                    mybir.AluOpType.add)
            nc.gpsimd.dma_start(out=io["outT"][d * P:(d + 1) * P, TS[th]], in_=osb)


_CACHED_NC = None


def build_nc():
    global _CACHED_NC
    if _CACHED_NC is not None:
        return _CACHED_NC
    nc = bacc.Bacc("TRN2", target_bir_lowering=False, debug=False,
                   enable_asserts=False, num_devices=NCORES)
    decls = [
        ("xT16", [D, N], BF16, False),
        ("dxT16", [D, N], BF16, False),
        ("cgT", [D, 80], BF16, False),
        ("biases", [P, 48], F32, False),
        ("bT", [ER, F], BF16, False),
        ("wiT", [D, F], BF16, False),
        ("woT", [F, D], BF16, False),
        ("outT", [D, N], F32, True),
    ]
    io = {}
    for name, shape, dt_, is_out in decls:
        io[name] = nc.dram_tensor(
            name, shape, dt_, kind="ExternalOutput" if is_out else "ExternalInput"
        ).ap()
    with tile.TileContext(nc) as tc:
        with ExitStack() as ctx:
            _emit(ctx, tc, io)
    nc.compile()
    _CACHED_NC = nc
    return nc


def make_in_maps(inputs: dict) -> list[dict]:
    f32 = np.float32
    x = np.ascontiguousarray(np.asarray(inputs["hidden_states"], f32).reshape(NT, D))
    gT = np.asarray(inputs["gate_W"], f32).T                                # [D, E]
    aT = np.asarray(inputs["lora_A"], f32).reshape(ER, D).T                 # [D, 32]
    ga = np.concatenate([aT, gT], axis=1)                                   # [D, 40]
    ga16 = ga.astype(BF)
    dga16 = (ga - ga16.astype(f32)).astype(BF)
    cgT = np.ascontiguousarray(np.concatenate([ga16, dga16], axis=1))       # [D, 80]
    biases = np.zeros((P, 48), f32)
    biases[:, 0:KF] = np.asarray(inputs["wi_b"], f32).reshape(KF, P).T
    biases[:, KF:KF + KD] = np.asarray(inputs["wo_b"], f32).reshape(KD, P).T
    biases[:, KF + KD:] = np.asarray(inputs["gate_b"], f32)[None, :]
    bT = np.ascontiguousarray(
        np.asarray(inputs["lora_B"], f32).transpose(0, 2, 1).reshape(ER, F).astype(BF))
    wiT = np.ascontiguousarray(np.asarray(inputs["wi_W"], f32).T.astype(BF))  # [D, F]
    woT = np.ascontiguousarray(np.asarray(inputs["wo_W"], f32).T.astype(BF))  # [F, D]

    in_maps = []
    for c in range(NCORES):
        xT32 = np.ascontiguousarray(x[c * N:(c + 1) * N].T)                 # [D, N]
        xT16 = xT32.astype(BF)
        dxT16 = (xT32 - xT16.astype(f32)).astype(BF)
        in_maps.append({
            "xT16": np.ascontiguousarray(xT16),
            "dxT16": np.ascontiguousarray(dxT16),
            "cgT": cgT, "biases": biases, "bT": bT,
            "wiT": wiT, "woT": woT,
        })
    return in_maps


def kernel(**inputs) -> np.ndarray:
    nc = build_nc()
    in_maps = make_in_maps(inputs)
    res = run_bass_kernel_spmd(nc, in_maps, core_ids=list(range(NCORES)))
    out = np.empty((NT, D), np.float32)
    for c in range(NCORES):
        out[c * N:(c + 1) * N] = res.results[c]["outT"].T
    return out.reshape(B, S, D)


# revision 21
# speedup vs baseline: 1.0817x; 1.0108x over previous
"""MoE block (top-1 routing, shared FFN + per-expert LoRA) on 8 TRN2 NeuronCores.

Strategy: data-parallel over the 8192 tokens (1024 tokens/core), weights
replicated. The reference's dense-then-mask expert loop collapses to:

    logits = x @ gate_W.T + gate_b ; e* = argmax(logits)        (fp32)
    u      = x @ A_cat.T                 [N, 32]                (bf16)
    u_m    = u * onehot-mask(e*)  (zero all but selected expert's 4 lora rows)
    inter  = relu(x @ wi_W.T + u_m @ B_cat + wi_b)              (bf16 matmul)
    out    = inter @ wo_W.T + wo_b                              (bf16 matmul)

Everything runs in transposed (feature-major) layout on chip; the host
pre-transposes the shards/weights and re-transposes the output.
"""

import numpy as np
import ml_dtypes
from contextlib import ExitStack

import concourse.bass as bass
import concourse.tile as tile
from concourse import bacc, mybir
from concourse.bass_utils import run_bass_kernel_spmd
from concourse.masks import make_identity

F32 = mybir.dt.float32
F32R = mybir.dt.float32r
BF16 = mybir.dt.bfloat16
U32 = mybir.dt.uint32
BF = ml_dtypes.bfloat16

B, S, D, F, E, R = 4, 2048, 1024, 4096, 8, 4
NCORES = 8
NT = B * S          # 8192 tokens total
N = NT // NCORES    # 1024 tokens per core
ER = E * R          # 32 lora rows
KD = D // 128       # 8 contraction tiles over D
KF = F // 128       # 32 contraction tiles over F
TT = N // 128       # 8 token tiles (routing)
TH = N // 512       # 2 token halves (matmul moving dim)
P = 128

Relu = mybir.ActivationFunctionType.Relu


def _emit(ctx: ExitStack, tc: tile.TileContext, io: dict):
    nc = tc.nc

    consts = ctx.enter_context(tc.tile_pool(name="consts", bufs=1))
    xpool = ctx.enter_context(tc.tile_pool(name="xpool", bufs=1))
    wipool = ctx.enter_context(tc.tile_pool(name="wipool", bufs=1))
    ipool = ctx.enter_context(tc.tile_pool(name="ipool", bufs=1))
    x32p = ctx.enter_context(tc.tile_pool(name="x32p", bufs=3))
    wop = ctx.enter_context(tc.tile_pool(name="wop", bufs=2))
    rwork = ctx.enter_context(tc.tile_pool(name="rwork", bufs=2))
    outp = ctx.enter_context(tc.tile_pool(name="outp", bufs=3))
    sps = ctx.enter_context(tc.tile_pool(name="sps", bufs=1, space="PSUM"))
    bps = ctx.enter_context(tc.tile_pool(name="bps", bufs=4, space="PSUM"))

    # ---------- constants ----------
    identity = consts.tile([P, P], F32, tag="identity")
    make_identity(nc, identity)
    identity16 = consts.tile([P, P], BF16, tag="identity16")
    nc.vector.tensor_copy(identity16, identity)
    # econst[p, e*R + r] = e  (expert id per lora row, replicated on free axis)
    # cols 32:128 hold an impossible id so the padded mask transposes to zeros
    econst = consts.tile([P, P], BF16, tag="econst")
    for e in range(E):
        nc.vector.memset(econst[:, e * R:(e + 1) * R], float(e))
    nc.vector.memset(econst[:, ER:], 255.0)
    # biases [128, 48] f32: cols 0:32 wi_b by f-tile, 32:40 wo_b by d-tile,
    # 40:48 gate_b replicated. cg [D, 80] bf16: cols 0:8 g16, 8:40 a16,
    # 40:48 dg16, 48:80 da16 (router+lora stationaries, fp32-split).
    biases_sb = consts.tile([P, 48], F32, tag="biases")
    nc.gpsimd.dma_start(out=biases_sb, in_=io["biases"])
    wib_sb = biases_sb[:, 0:KF]
    wob_sb = biases_sb[:, KF:KF + KD]
    gateb_sb = biases_sb[:, KF + KD:KF + KD + E]
    cg_big = consts.tile([P, KD * 80], BF16, tag="cg")
    nc.gpsimd.dma_start(out=cg_big.rearrange("p (k c) -> p k c", k=KD),
                        in_=io["cgT"].rearrange("(k p) c -> p k c", p=P))
    cg_sb = [cg_big[:, k * 80:(k + 1) * 80] for k in range(KD)]
    bcat_sb = consts.tile([ER, F], BF16, tag="bc")
    nc.gpsimd.dma_start(out=bcat_sb, in_=io["bT"])

    # ---------- PE warm-up: release the HAM clock gate while DMAs land ----------
    warm_src = consts.tile([P, 512], BF16, tag="warm")
    nc.vector.memset(warm_src, 1.0)
    for w in range(24):
        psum_w = bps.tile([P, 512], F32, tag="pbig", name=f"pw{w}")
        nc.tensor.matmul(psum_w, lhsT=warm_src[:, 0:P], rhs=warm_src,
                         start=True, stop=True)

    # ---------- resident activations / weights ----------
    inter_sb = [ipool.tile([P, N], BF16, tag=f"inter{f}", name=f"inter{f}")
                for f in range(KF)]
    maskT4 = consts.tile([ER, N], BF16, tag="maskT4")
    um16 = [consts.tile([ER, 512], BF16, tag=f"um{th}", name=f"um{th}")
            for th in range(TH)]

    # ---------- DMA priority order on the sync queue:
    #   x16 -> dx16 (router-critical) -> wi halves -> wo; consts + outs on
    #   the gpsimd queue. Consolidated 3D-AP DMAs to cut issue serialization.
    QF = F // 4   # 1024 f-columns per wi quarter
    HK = KD // 2
    x16h = []
    dx16h = []
    x16_src = io["xT16"].rearrange("(h k p) t -> h p k t", h=2, p=P)
    dx16_src = io["dxT16"].rearrange("(h k p) t -> h p k t", h=2, p=P)
    for h in range(2):
        t = xpool.tile([P, HK * N], BF16, tag=f"x16h{h}", name=f"x16h{h}")
        nc.sync.dma_start(out=t.rearrange("p (k t) -> p k t", k=HK),
                          in_=x16_src[h])
        x16h.append(t)
    for h in range(2):
        t = x32p.tile([P, HK * N], BF16, tag=f"dx16h{h}", name=f"dx16h{h}",
                      bufs=1)
        nc.sync.dma_start(out=t.rearrange("p (k t) -> p k t", k=HK),
                          in_=dx16_src[h])
        dx16h.append(t)
    x16 = [x16h[k // HK][:, (k % HK) * N:(k % HK + 1) * N] for k in range(KD)]
    dx16 = [dx16h[k // HK][:, (k % HK) * N:(k % HK + 1) * N] for k in range(KD)]
    wi_src = io["wiT"].rearrange("(k p) f -> p k f", p=P)
    wi_q = []
    for q in range(4):
        wq = wipool.tile([P, KD * QF], BF16, tag=f"wiq{q}", name=f"wiq{q}")
        nc.sync.dma_start(out=wq.rearrange("p (k f) -> p k f", k=KD),
                          in_=wi_src[:, :, q * QF:(q + 1) * QF])
        wi_q.append(wq)

    def wi_lhsT(k, f):
        q, fr = divmod(f, 8)
        return wi_q[q][:, k * QF + fr * P:k * QF + (fr + 1) * P]

    # ---------- router + lora projection, one fused group ----------
    # [logits | u] = x@[g | Acat] via 3 bf16 terms (fp32-accurate):
    #   x16@(g16|a16) + dx16@(g16|a16) + x16@(dg16|da16)
    TS = [slice(th * 512, (th + 1) * 512) for th in range(TH)]
    psum_cu = [sps.tile([E + ER, 512], F32, tag=f"pcu{th}", name=f"pcu{th}")
               for th in range(TH)]
    gbrow = consts.tile([1, 40], BF16, tag="gbrow")
    nc.vector.memset(gbrow[:, 0:ER], 0.0)
    nc.vector.tensor_copy(gbrow[:, ER:40], gateb_sb[0:1, :])
    ones_row = consts.tile([1, 512], BF16, tag="ones_row")
    nc.vector.memset(ones_row, 1.0)
    phases = [(0, x16), (40, x16), (0, dx16)]
    for pi, (col, xs) in enumerate(phases):
        for k in range(KD):
            for th in range(TH):
                nc.tensor.matmul(psum_cu[th], lhsT=cg_sb[k][:, col:col + 40],
                                 rhs=xs[k][:, TS[th]],
                                 start=(pi == 0 and k == 0), stop=False)
    for th in range(TH):
        nc.tensor.matmul(psum_cu[th], lhsT=gbrow, rhs=ones_row,
                         start=False, stop=True)
    for th in range(TH):
        logitsT = rwork.tile([E, 512], F32, tag="lgT")
        nc.vector.tensor_copy(logitsT, psum_cu[th][ER:ER + E, :])
        for q in range(4):
            tt = th * 4 + q
            # transpose [8, 128] logit chunk to token-major [128, 8]
            psum_tr = sps.tile([P, E], F32, tag="pmask", name=f"ptr{tt}", bufs=2)
            nc.tensor.matmul(psum_tr, lhsT=logitsT[:, q * P:(q + 1) * P],
                             rhs=identity[0:E, 0:E], is_transpose=True,
                             start=True, stop=True)
            logits = rwork.tile([P, E], F32, tag="lg")
            nc.vector.tensor_copy(logits, psum_tr)
            max8 = rwork.tile([P, E], F32, tag="mx")
            nc.vector.max(out=max8, in_=logits)
            idx8 = rwork.tile([P, E], U32, tag="ix")
            nc.vector.max_index(idx8, max8, logits)
            idxf = rwork.tile([P, 1], F32, tag="if")
            nc.vector.tensor_copy(idxf, idx8[:, 0:1])
            # mask_rep[t, e*R+r] = (argmax == e); bf16 is exact for 0/1
            mask_rep = rwork.tile([P, ER], BF16, tag="mr")
            nc.vector.tensor_scalar(mask_rep, econst[:, 0:ER], idxf, None,
                                    mybir.AluOpType.is_equal)
            psum_m = sps.tile([ER, P], BF16, tag="pmask", name=f"pm{tt}",
                              bufs=2)
            nc.tensor.matmul(psum_m, lhsT=mask_rep, rhs=identity16,
                             is_transpose=True, start=True, stop=True)
            nc.vector.tensor_copy(maskT4[:, tt * P:(tt + 1) * P], psum_m)

    # ---------- masked lora projection from the fused psum ----------
    for th in range(TH):
        nc.vector.tensor_mul(um16[th], psum_cu[th][0:ER, :],
                             maskT4[:, TS[th]])

    # ---------- matmul 1: interT = relu(wi @ x.T + Bcat.T @ u_m + wi_b) ------
    # th0/th1 paired per stationary tile so each weight load feeds 2 matmuls
    for f in range(KF):
        ps = [bps.tile([P, 512], F32, tag="pbig", name=f"p1_{f}_{th}")
              for th in range(TH)]
        for k in range(KD):
            for th in range(TH):
                nc.tensor.matmul(ps[th], lhsT=wi_lhsT(k, f),
                                 rhs=x16[k][:, TS[th]],
                                 start=(k == 0), stop=False)
        for th in range(TH):
            nc.tensor.matmul(ps[th], lhsT=bcat_sb[:, f * P:(f + 1) * P],
                             rhs=um16[th], start=False, stop=True)
        for th in range(TH):
            nc.scalar.activation(inter_sb[f][:, TS[th]], ps[th], Relu,
                                 bias=wib_sb[:, f:f + 1])

    # ---------- matmul 2: outT = wo @ inter + wo_b ----------
    # woT [F, D] column-block d fetched as ONE 3D DMA into [p, (kf j)] layout:
    # wo_big[p, kf*128 + j] = woT[kf*128 + p, d*128 + j]
    wo_src = io["woT"].rearrange("(kf p) d -> p kf d", p=P)
    for d in range(KD):
        wo_big = wop.tile([P, F], BF16, tag="wo", name=f"wo{d}")
        nc.sync.dma_start(out=wo_big.rearrange("p (kf j) -> p kf j", kf=KF),
                          in_=wo_src[:, :, d * P:(d + 1) * P])
        ps = [bps.tile([P, 512], F32, tag="pbig", name=f"p2_{d}_{th}")
              for th in range(TH)]
        for kf in range(KF):
            for th in range(TH):
                nc.tensor.matmul(ps[th], lhsT=wo_big[:, kf * P:(kf + 1) * P],
                                 rhs=inter_sb[kf][:, TS[th]],
                                 start=(kf == 0), stop=(kf == KF - 1))
        for th in range(TH):
            osb = outp.tile([P, 512], F32, tag="osb")
            nc.vector.tensor_scalar(osb, ps[th], wob_sb[:, d:d + 1], None,
                                    mybir.AluOpType.add)
            nc.gpsimd.dma_start(out=io["outT"][d * P:(d + 1) * P, TS[th]], in_=osb)


_CACHED_NC = None


def build_nc():
    global _CACHED_NC
    if _CACHED_NC is not None:
        return _CACHED_NC
    nc = bacc.Bacc("TRN2", target_bir_lowering=False, debug=False,
                   enable_asserts=False, num_devices=NCORES)
    decls = [
        ("xT16", [D, N], BF16, False),
        ("dxT16", [D, N], BF16, False),
        ("cgT", [D, 80], BF16, False),
        ("biases", [P, 48], F32, False),
        ("bT", [ER, F], BF16, False),
        ("wiT", [D, F], BF16, False),
        ("woT", [F, D], BF16, False),
        ("outT", [D, N], F32, True),
    ]
    io = {}
    for name, shape, dt_, is_out in decls:
        io[name] = nc.dram_tensor(
            name, shape, dt_, kind="ExternalOutput" if is_out else "ExternalInput"
        ).ap()
    with tile.TileContext(nc) as tc:
        with ExitStack() as ctx:
            _emit(ctx, tc, io)
    nc.compile()
    _CACHED_NC = nc
    return nc


def make_in_maps(inputs: dict) -> list[dict]:
    f32 = np.float32
    x = np.ascontiguousarray(np.asarray(inputs["hidden_states"], f32).reshape(NT, D))
    gT = np.asarray(inputs["gate_W"], f32).T                                # [D, E]
    aT = np.asarray(inputs["lora_A"], f32).reshape(ER, D).T                 # [D, 32]
    ga = np.concatenate([aT, gT], axis=1)                                   # [D, 40]
    ga16 = ga.astype(BF)
    dga16 = (ga - ga16.astype(f32)).astype(BF)
    cgT = np.ascontiguousarray(np.concatenate([ga16, dga16], axis=1))       # [D, 80]
    biases = np.zeros((P, 48), f32)
    biases[:, 0:KF] = np.asarray(inputs["wi_b"], f32).reshape(KF, P).T
    biases[:, KF:KF + KD] = np.asarray(inputs["wo_b"], f32).reshape(KD, P).T
    biases[:, KF + KD:] = np.asarray(inputs["gate_b"], f32)[None, :]
    bT = np.ascontiguousarray(
        np.asarray(inputs["lora_B"], f32).transpose(0, 2, 1).reshape(ER, F).astype(BF))
    wiT = np.ascontiguousarray(np.asarray(inputs["wi_W"], f32).T.astype(BF))  # [D, F]
    woT = np.ascontiguousarray(np.asarray(inputs["wo_W"], f32).T.astype(BF))  # [F, D]

    in_maps = []
    for c in range(NCORES):
        xT32 = np.ascontiguousarray(x[c * N:(c + 1) * N].T)                 # [D, N]
        xT16 = xT32.astype(BF)
        dxT16 = (xT32 - xT16.astype(f32)).astype(BF)
        in_maps.append({
            "xT16": np.ascontiguousarray(xT16),
            "dxT16": np.ascontiguousarray(dxT16),
            "cgT": cgT, "biases": biases, "bT": bT,
            "wiT": wiT, "woT": woT,
        })
    return in_maps


def kernel(**inputs) -> np.ndarray:
    nc = build_nc()
    in_maps = make_in_maps(inputs)
    res = run_bass_kernel_spmd(nc, in_maps, core_ids=list(range(NCORES)))
    out = np.empty((NT, D), np.float32)
    for c in range(NCORES):
        out[c * N:(c + 1) * N] = res.results[c]["outT"].T
    return out.reshape(B, S, D)
